# revision 3
# baseline (speedup 1.0000x reference)
"""GrwSmoothingLoss on 8 Trainium2 NeuronCores — v2.

Math: logits[b,p] = -0.5*||diff2(Z_b[perm_p])||^2 = <G_b, -0.5*C_p> where
G_b = Z_b Z_b^T (8x8 Gram) and C_p = M_p^T M_p with M_p the permuted
second-difference matrix.  alpha*V_b = <G_b, 0.25*C1>.  C_p entries are
small integers, so the coefficient matrix callT = [-0.5*C_p | 0.25*C1] is
built on host from perm_index and is EXACT in fp16.

Per core (32 batches, data-parallel over B):
  DMA in: zbq [128,288] fp16 (rows (b,q): Z[b,:,q*32:(q+1)*32] as (t,k) cols
          0..255, one-hot q-fold matrix cols 256..287) and callT [64,1002]
          fp16 (two 501-col chunks: 500 perm cols + the alpha*V col).
  Gram:   pair products on DVE (fp16 2x) + Pool slice, two binary k-folds,
          one TensorReduce -> gq [128,64]; PE folds q via the one-hot matmul
          -> G^T [64,32].
  Logits: two 501-col fp16 matmuls -> PSUM [64,501] (chunk c in partitions
          c*32..c*32+32).  One Exp over [64,500] with accum_out -> partial
          sums of exp(logit).  Host does ln / means (fp64).
"""

import numpy as np

import concourse.bacc as bacc
import concourse.bass as bass
import concourse.mybir as mybir
import concourse.tile as tile
from concourse.bass_utils import run_bass_kernel_spmd

B, T, K = 256, 8, 128
NUM_PERMS = 1000
ALPHA = 0.5
N_CORES = 8
B_LOC = B // N_CORES  # 32
HALF = NUM_PERMS // 2  # 500
CCOL = HALF + 1  # 501 cols per chunk (500 perms + aV col)
F32 = mybir.dt.float32
F16 = mybir.dt.float16

_cache = {}


def _kernel_body(tc, out_d, zbq_d, callT_d):
    nc = tc.nc
    with (
        tc.tile_pool(name="sb", bufs=1) as sb,
        tc.tile_pool(name="ps", bufs=1, space="PSUM") as ps,
    ):
        # dummy Exp to pull the activation table load into the DMA window
        dm0 = sb.tile([1, 1], F32)
        dm1 = sb.tile([1, 1], F32)
        nc.gpsimd.memset(dm0[:], 0.0)
        nc.scalar.activation(dm1[:], dm0[:], mybir.ActivationFunctionType.Exp)

        zbq = sb.tile([128, 288], F16)
        callT = sb.tile([64, 2 * CCOL], F16)
        nc.sync.dma_start(out=zbq[:], in_=zbq_d[:])
        nc.sync.dma_start(out=callT[:], in_=callT_d[:])

        zv = zbq[:, 0:256].rearrange("p (t k) -> p t k", t=8)

        # pair products pp[(b,q),(i,j,k)] = z_i[k] * z_j[k], fp16
        pp = sb.tile([128, 2048], F16)
        ppv = pp[:].rearrange("p (i j k) -> p i j k", i=8, j=8)
        nc.gpsimd.tensor_tensor(
            out=ppv[:, 0:1, :, :],
            in0=zv[:, 0:1, :].unsqueeze(2).broadcast_to([128, 1, 8, 32]),
            in1=zv.unsqueeze(1).broadcast_to([128, 1, 8, 32]),
            op=mybir.AluOpType.mult,
        )
        nc.vector.tensor_tensor(
            out=ppv[:, 1:8, :, :],
            in0=zv[:, 1:8, :].unsqueeze(2).broadcast_to([128, 7, 8, 32]),
            in1=zv.unsqueeze(1).broadcast_to([128, 7, 8, 32]),
            op=mybir.AluOpType.mult,
        )

        # binary k-folds 32 -> 16 -> 8, then reduce -> gq [128, 64]
        ppk = pp[:].rearrange("p (ij k) -> p ij k", k=32)
        pf1 = sb.tile([128, 1024], F16)
        pf1v = pf1[:].rearrange("p (ij k) -> p ij k", k=16)
        nc.vector.tensor_tensor(
            out=pf1v, in0=ppk[:, :, 0:16], in1=ppk[:, :, 16:32],
            op=mybir.AluOpType.add,
        )
        pf2 = sb.tile([128, 512], F16)
        pf2v = pf2[:].rearrange("p (ij k) -> p ij k", k=8)
        nc.vector.tensor_tensor(
            out=pf2v, in0=pf1v[:, :, 0:8], in1=pf1v[:, :, 8:16],
            op=mybir.AluOpType.add,
        )
        gq = sb.tile([128, 64], F16)
        with nc.allow_low_precision(reason="8-term fp16 fold of unit-norm rows"):
            nc.vector.reduce_sum(out=gq[:], in_=pf2v, axis=mybir.AxisListType.X)

        # fold q via one-hot matmul: G^T [64=(i,j), 32=b]
        psum_g = ps.tile([64, B_LOC], F32)
        nc.tensor.matmul(psum_g[:], gq[:], zbq[:, 256:288])
        gT = sb.tile([64, B_LOC], F16)
        nc.vector.tensor_copy(gT[:], psum_g[:])

        # logits: chunk c -> psum partitions c*32..c*32+32
        psum_X = ps.tile([64, CCOL], F32)
        nc.tensor.matmul(psum_X[0:B_LOC, :], gT[:], callT[:, 0:CCOL])
        nc.tensor.matmul(psum_X[B_LOC : 2 * B_LOC, :], gT[:], callT[:, CCOL : 2 * CCOL])

        # out cols: 0 = sum_p exp(logit) (chunk partials), 1 = logit0, 2 = aV
        out_t = sb.tile([64, 3], F32)
        nc.vector.tensor_copy(out_t[:, 1:2], psum_X[:, 0:1])
        nc.vector.tensor_copy(out_t[:, 2:3], psum_X[:, HALF : HALF + 1])
        e = sb.tile([64, HALF], F16)
        nc.scalar.activation(
            e[:], psum_X[:, 0:HALF], mybir.ActivationFunctionType.Exp,
            accum_out=out_t[:, 0:1],
        )
        nc.sync.dma_start(out=out_d[:], in_=out_t[:])


def _build():
    if "nc" in _cache:
        return _cache["nc"]
    nc = bacc.Bacc(
        "TRN2",
        target_bir_lowering=False,
        debug=False,
        enable_asserts=False,
        num_devices=N_CORES,
    )
    zbq_d = nc.dram_tensor("zbq", [128, 288], F16, kind="ExternalInput").ap()
    callT_d = nc.dram_tensor("callT", [64, 2 * CCOL], F16, kind="ExternalInput").ap()
    out_d = nc.dram_tensor("out_t", [64, 3], F32, kind="ExternalOutput").ap()
    with tile.TileContext(nc) as tc:
        _kernel_body(tc, out_d, zbq_d, callT_d)
    nc.compile()
    _cache["nc"] = nc
    return nc


def _coeffs(perm_index):
    """callT [64, 2*CCOL] fp16: rows (i,j); chunk c cols = [-0.5*C_p for the
    chunk's 500 perms, 0.25*C1].  All entries are small ints or quarters —
    exact in fp16."""
    perm = np.asarray(perm_index, dtype=np.int64).reshape(NUM_PERMS, T)
    E = (perm[:, :, None] == np.arange(T)[None, None, :]).astype(np.float32)
    M = E[:, 0:6] - 2.0 * E[:, 1:7] + E[:, 2:8]  # [P, 6, 8]
    C = np.einsum("pri,prj->pij", M, M)  # [P, 8, 8]
    D1 = (np.eye(T, k=1) - np.eye(T))[: T - 1]
    C1 = (D1.T @ D1).astype(np.float32)
    callT = np.zeros((T * T, 2 * CCOL), np.float32)
    for c in range(2):
        cols = (-0.5 * C[c * HALF : (c + 1) * HALF]).reshape(HALF, 64).T
        callT[:, c * CCOL : c * CCOL + HALF] = cols
        callT[:, c * CCOL + HALF] = (0.25 * C1).reshape(64)
    return callT.astype(np.float16)


def _in_maps(Z, perm_index):
    callT = _coeffs(perm_index)
    q4 = np.repeat(np.eye(B_LOC, dtype=np.float16), 4, axis=0)  # [128, 32]
    Zf = np.asarray(Z, dtype=np.float32).reshape(B, T, 4, 32)
    in_maps = []
    for c in range(N_CORES):
        zb = (
            np.ascontiguousarray(Zf[c * B_LOC : (c + 1) * B_LOC].transpose(0, 2, 1, 3))
            .reshape(128, 256)
            .astype(np.float16)
        )
        zbq = np.concatenate([zb, q4], axis=1)  # [128, 288]
        in_maps.append({"zbq": zbq, "callT": callT})
    return in_maps


def kernel(Z, perm_index, _trace=False):
    nc = _build()
    in_maps = _in_maps(Z, perm_index)
    res = run_bass_kernel_spmd(
        nc, in_maps, core_ids=list(range(N_CORES)), trace=_trace
    )
    total = np.float64(0.0)
    for r in res.results:
        o = np.asarray(r["out_t"], dtype=np.float64)
        s = o[0:B_LOC, 0] + o[B_LOC : 2 * B_LOC, 0]
        logit0 = o[0:B_LOC, 1]
        aV = o[0:B_LOC, 2]
        total += np.sum(np.log(s) - logit0 + aV)
    out = np.array(total / B, dtype=np.float32)
    if _trace:
        return out, res
    return out


# revision 5
# speedup vs baseline: 954.4621x; 954.4621x over previous
"""GrwSmoothingLoss on 8 Trainium2 NeuronCores — v2.

Math: logits[b,p] = -0.5*||diff2(Z_b[perm_p])||^2 = <G_b, -0.5*C_p> where
G_b = Z_b Z_b^T (8x8 Gram) and C_p = M_p^T M_p with M_p the permuted
second-difference matrix.  alpha*V_b = <G_b, 0.25*C1>.  C_p entries are
small integers, so the coefficient matrix callT = [-0.5*C_p | 0.25*C1] is
built on host from perm_index and is EXACT in fp16.

Per core (32 batches, data-parallel over B):
  DMA in: zbq [128,288] fp16 (rows (b,q): Z[b,:,q*32:(q+1)*32] as (t,k) cols
          0..255, one-hot q-fold matrix cols 256..287) and callT [64,1002]
          fp16 (two 501-col chunks: 500 perm cols + the alpha*V col).
  Gram:   pair products on DVE (fp16 2x) + Pool slice, two binary k-folds,
          one TensorReduce -> gq [128,64]; PE folds q via the one-hot matmul
          -> G^T [64,32].
  Logits: two 501-col fp16 matmuls -> PSUM [64,501] (chunk c in partitions
          c*32..c*32+32).  One Exp over [64,500] with accum_out -> partial
          sums of exp(logit).  Host does ln / means (fp64).
"""

import numpy as np

import concourse.bacc as bacc
import concourse.bass as bass
import concourse.mybir as mybir
import concourse.tile as tile
from concourse.bass_utils import run_bass_kernel_spmd

B, T, K = 256, 8, 128
NUM_PERMS = 1000
ALPHA = 0.5
N_CORES = 8
B_LOC = B // N_CORES  # 32
HALF = NUM_PERMS // 2  # 500
CCOL = HALF + 1  # 501 cols per chunk (500 perms + aV col)
F32 = mybir.dt.float32
F16 = mybir.dt.float16

_cache = {}


def _kernel_body(tc, out_d, zbq_d, callT_d, chain=None, first=True):
    nc = tc.nc
    with (
        tc.tile_pool(name="sb", bufs=1) as sb,
        tc.tile_pool(name="ps", bufs=1, space="PSUM") as ps,
    ):
        if first:
            # dummy Exp to pull the activation table load into the DMA window
            dm0 = sb.tile([1, 1], F32)
            dm1 = sb.tile([1, 1], F32)
            nc.gpsimd.memset(dm0[:], 0.0)
            nc.scalar.activation(dm1[:], dm0[:], mybir.ActivationFunctionType.Exp)

        zbq = sb.tile([128, 288], F16)
        if chain is not None and not first:
            # serialize bench iterations: WAW on zbq forces this iter's DMA
            # to wait for the previous iter's tail
            nc.vector.tensor_copy(zbq[0:1, 0:1], chain[:])
        callT = sb.tile([64, 2 * HALF], F16)
        nc.sync.dma_start(out=zbq[:], in_=zbq_d[:])
        nc.sync.dma_start(out=callT[:], in_=callT_d[:])

        zv = zbq[:, 0:256].rearrange("p (t k) -> p t k", t=8)

        # pair products pp[(b,q),(i,j,k)] = z_i[k] * z_j[k], fp16.
        # Pool (3.7x slower/col than DVE-fp16) takes the i in {0,1} block as
        # one op plus the matching fold1 slice; DVE owns the rest + the
        # X-reduce (DVE-only hardware).
        pp = sb.tile([128, 2048], F16)
        ppv = pp[:].rearrange("p (i j k) -> p i j k", i=8, j=8)
        nc.gpsimd.tensor_tensor(
            out=ppv[:, 0:1, :, :],
            in0=zv[:, 0:1, :].unsqueeze(2).broadcast_to([128, 1, 8, 32]),
            in1=zv.unsqueeze(1).broadcast_to([128, 1, 8, 32]),
            op=mybir.AluOpType.mult,
        )
        nc.vector.tensor_tensor(
            out=ppv[:, 1:8, :, :],
            in0=zv[:, 1:8, :].unsqueeze(2).broadcast_to([128, 7, 8, 32]),
            in1=zv.unsqueeze(1).broadcast_to([128, 7, 8, 32]),
            op=mybir.AluOpType.mult,
        )

        # binary k-folds 32 -> 16 -> 8, then reduce -> gq [128, 64]
        ppk = pp[:].rearrange("p (ij k) -> p ij k", k=32)
        pf1 = sb.tile([128, 1024], F16)
        pf1v = pf1[:].rearrange("p (ij k) -> p ij k", k=16)
        pf2 = sb.tile([128, 512], F16)
        pf2v = pf2[:].rearrange("p (ij k) -> p ij k", k=8)
        nc.gpsimd.tensor_tensor(
            out=pf1v[:, 0:8, :], in0=ppk[:, 0:8, 0:16],
            in1=ppk[:, 0:8, 16:32], op=mybir.AluOpType.add,
        )
        nc.gpsimd.tensor_tensor(
            out=pf2v[:, 0:8, :], in0=pf1v[:, 0:8, 0:8],
            in1=pf1v[:, 0:8, 8:16], op=mybir.AluOpType.add,
        )
        nc.vector.tensor_tensor(
            out=pf1v[:, 8:64, :], in0=ppk[:, 8:64, 0:16],
            in1=ppk[:, 8:64, 16:32], op=mybir.AluOpType.add,
        )
        nc.vector.tensor_tensor(
            out=pf2v[:, 8:64, :], in0=pf1v[:, 8:64, 0:8],
            in1=pf1v[:, 8:64, 8:16], op=mybir.AluOpType.add,
        )
        gq = sb.tile([128, 64], F16)
        with nc.allow_low_precision(reason="8-term fp16 fold of unit-norm rows"):
            nc.vector.reduce_sum(out=gq[:], in_=pf2v, axis=mybir.AxisListType.X)

        # fold q via one-hot matmul: G^T [64=(i,j), 32=b]
        psum_g = ps.tile([64, B_LOC], F32)
        nc.tensor.matmul(psum_g[:], gq[:], zbq[:, 256:288])
        gT = sb.tile([64, B_LOC], F16)
        nc.vector.tensor_copy(gT[:], psum_g[:])

        # out cols 0,1 = per-chunk sum_p exp(logit) (rows 0..31); cols 2..34 =
        # G^T in f32 (host derives logit0 and the alpha*V term from G)
        out_t = sb.tile([64, 2 + B_LOC], F32)
        nc.gpsimd.memset(out_t[B_LOC : 2 * B_LOC, 0:2], 0.0)
        nc.vector.tensor_copy(out_t[:, 2 : 2 + B_LOC], psum_g[:])

        # logits: one PSUM tile per chunk so exp(chunk0) overlaps matmul(chunk1)
        psum_X0 = ps.tile([B_LOC, HALF], F32)
        psum_X1 = ps.tile([B_LOC, HALF], F32)
        nc.tensor.matmul(psum_X0[:], gT[:], callT[:, 0:HALF])
        nc.tensor.matmul(psum_X1[:], gT[:], callT[:, HALF : 2 * HALF])

        e0 = sb.tile([B_LOC, HALF], F16)
        e1 = sb.tile([B_LOC, HALF], F16)
        nc.scalar.activation(
            e0[:], psum_X0[:], mybir.ActivationFunctionType.Exp,
            accum_out=out_t[0:B_LOC, 0:1],
        )
        nc.scalar.activation(
            e1[:], psum_X1[:], mybir.ActivationFunctionType.Exp,
            accum_out=out_t[0:B_LOC, 1:2],
        )
        if chain is not None:
            nc.vector.tensor_copy(chain[:], out_t[0:1, 0:1])
        nc.sync.dma_start(out=out_d[:], in_=out_t[:])


def _build(nloop=1):
    key = ("nc", nloop)
    if key in _cache:
        return _cache[key]
    nc = bacc.Bacc(
        "TRN2",
        target_bir_lowering=False,
        debug=False,
        enable_asserts=False,
        num_devices=N_CORES,
    )
    zbq_d = nc.dram_tensor("zbq", [128, 288], F16, kind="ExternalInput").ap()
    callT_d = nc.dram_tensor("callT", [64, 2 * HALF], F16, kind="ExternalInput").ap()
    out_d = nc.dram_tensor("out_t", [64, 2 + B_LOC], F32, kind="ExternalOutput").ap()
    with tile.TileContext(nc) as tc:
        if nloop == 1:
            _kernel_body(tc, out_d, zbq_d, callT_d)
        else:
            with tc.tile_pool(name="chain", bufs=1) as cp:
                chain = cp.tile([1, 1], F32)
                for i in range(nloop):
                    _kernel_body(
                        tc, out_d, zbq_d, callT_d, chain=chain, first=(i == 0)
                    )
    nc.compile()
    _cache[key] = nc
    return nc


def _coeffs(perm_index):
    """callT [64, 1000] fp16: rows (i,j), col p = -0.5*C_p.  All entries are
    small ints or halves — exact in fp16.  Also returns -0.5*C_0 and 0.25*C1
    (f64) for the host-side logit0 / alpha*V terms."""
    perm = np.asarray(perm_index, dtype=np.int64).reshape(NUM_PERMS, T)
    E = (perm[:, :, None] == np.arange(T)[None, None, :]).astype(np.float32)
    M = E[:, 0:6] - 2.0 * E[:, 1:7] + E[:, 2:8]  # [P, 6, 8]
    C = np.einsum("pri,prj->pij", M, M)  # [P, 8, 8]
    D1 = (np.eye(T, k=1) - np.eye(T))[: T - 1]
    C1 = (D1.T @ D1).astype(np.float64)
    callT = (-0.5 * C).reshape(NUM_PERMS, 64).T.astype(np.float16)
    return np.ascontiguousarray(callT), (-0.5 * C[0]).astype(np.float64), 0.25 * C1


def _in_maps(Z, perm_index):
    callT, _, _ = _coeffs(perm_index)
    q4 = np.repeat(np.eye(B_LOC, dtype=np.float16), 4, axis=0)  # [128, 32]
    Zf = np.asarray(Z, dtype=np.float32).reshape(B, T, 4, 32)
    in_maps = []
    for c in range(N_CORES):
        zb = (
            np.ascontiguousarray(Zf[c * B_LOC : (c + 1) * B_LOC].transpose(0, 2, 1, 3))
            .reshape(128, 256)
            .astype(np.float16)
        )
        zbq = np.concatenate([zb, q4], axis=1)  # [128, 288]
        in_maps.append({"zbq": zbq, "callT": callT})
    return in_maps


def kernel(Z, perm_index, _trace=False):
    nc = _build()
    in_maps = _in_maps(Z, perm_index)
    _, c0, c1q = _coeffs(perm_index)
    res = run_bass_kernel_spmd(
        nc, in_maps, core_ids=list(range(N_CORES)), trace=_trace
    )
    total = np.float64(0.0)
    for r in res.results:
        o = np.asarray(r["out_t"], dtype=np.float64)
        s = o[0:B_LOC, 0] + o[0:B_LOC, 1]
        # G^T [64=(i,j), 32=b] in f32: host computes logit0 and alpha*V
        gT = o[:, 2 : 2 + B_LOC]
        logit0 = c0.reshape(64) @ gT
        aV = c1q.reshape(64) @ gT
        total += np.sum(np.log(s) - logit0 + aV)
    out = np.array(total / B, dtype=np.float32)
    if _trace:
        return out, res
    return out


# revision 6
# speedup vs baseline: 40242.1739x; 42.1622x over previous
"""GrwSmoothingLoss on 8 Trainium2 NeuronCores.

Math: logits[b,p] = -0.5*||diff2(Z_b[perm_p])||^2 = <G_b, -0.5*C_p> where
G_b = Z_b Z_b^T (8x8 Gram) and C_p = M_p^T M_p with M_p the permuted
second-difference matrix; alpha*V_b = <G_b, 0.25*C1>.  C_p entries are
small integers, so the host-built coefficient matrix callT = -0.5*C_p is
EXACT in fp16.

Per core (32 batches, data-parallel over B):
  DMA in: zbq [128,288] fp16 (rows (b,q): Z[b,:,q*32:(q+1)*32] as (t,k) cols
          0..255, one-hot q-fold matrix cols 256..287) and callT [64,1000]
          fp16 (two 500-perm chunks).
  Gram:   pair products z_i*z_j on DVE (fp16 2x mode) with the i=0 block +
          its fold slices on Pool, two binary k-folds, one TensorReduce ->
          gq [128,64]; PE folds the 4-way k-split via the one-hot matmul ->
          G^T [64,32] (also shipped out in f32 for host-side logit0 / V).
  Logits: two 500-col fp16 matmuls into separate PSUM banks so Exp(chunk0)
          overlaps matmul(chunk1); each Exp's accum_out yields the chunk's
          sum of exp(logit).  Host (fp64) does ln, logit0, alpha*V, means.
"""

import numpy as np

import concourse.bacc as bacc
import concourse.bass as bass
import concourse.mybir as mybir
import concourse.tile as tile
from concourse.bass_utils import run_bass_kernel_spmd

B, T, K = 256, 8, 128
NUM_PERMS = 1000
ALPHA = 0.5
N_CORES = 8
B_LOC = B // N_CORES  # 32
HALF = NUM_PERMS // 2  # 500
CCOL = HALF + 1  # 501 cols per chunk (500 perms + aV col)
F32 = mybir.dt.float32
F16 = mybir.dt.float16

_cache = {}


def _kernel_body(tc, out_d, zbq_d, callT_d, chain=None, first=True):
    nc = tc.nc
    with (
        tc.tile_pool(name="sb", bufs=1) as sb,
        tc.tile_pool(name="ps", bufs=1, space="PSUM") as ps,
    ):
        if first:
            # dummy Exp to pull the activation table load into the DMA window
            dm0 = sb.tile([1, 1], F32)
            dm1 = sb.tile([1, 1], F32)
            nc.gpsimd.memset(dm0[:], 0.0)
            nc.scalar.activation(dm1[:], dm0[:], mybir.ActivationFunctionType.Exp)

        zbq = sb.tile([128, 288], F16)
        if chain is not None and not first:
            # serialize bench iterations: WAW on zbq forces this iter's DMA
            # to wait for the previous iter's tail
            nc.vector.tensor_copy(zbq[0:1, 0:1], chain[:])
        callT = sb.tile([64, 2 * HALF], F16)
        nc.sync.dma_start(out=zbq[:], in_=zbq_d[:])
        nc.sync.dma_start(out=callT[:], in_=callT_d[:])

        zv = zbq[:, 0:256].rearrange("p (t k) -> p t k", t=8)

        # pair products pp[(b,q),(i,j,k)] = z_i[k] * z_j[k], fp16.
        # Pool (3.7x slower/col than DVE-fp16) takes the i in {0,1} block as
        # one op plus the matching fold1 slice; DVE owns the rest + the
        # X-reduce (DVE-only hardware).
        pp = sb.tile([128, 2048], F16)
        ppv = pp[:].rearrange("p (i j k) -> p i j k", i=8, j=8)
        nc.gpsimd.tensor_tensor(
            out=ppv[:, 0:1, :, :],
            in0=zv[:, 0:1, :].unsqueeze(2).broadcast_to([128, 1, 8, 32]),
            in1=zv.unsqueeze(1).broadcast_to([128, 1, 8, 32]),
            op=mybir.AluOpType.mult,
        )
        nc.vector.tensor_tensor(
            out=ppv[:, 1:8, :, :],
            in0=zv[:, 1:8, :].unsqueeze(2).broadcast_to([128, 7, 8, 32]),
            in1=zv.unsqueeze(1).broadcast_to([128, 7, 8, 32]),
            op=mybir.AluOpType.mult,
        )

        # binary k-folds 32 -> 16 -> 8, then reduce -> gq [128, 64]
        ppk = pp[:].rearrange("p (ij k) -> p ij k", k=32)
        pf1 = sb.tile([128, 1024], F16)
        pf1v = pf1[:].rearrange("p (ij k) -> p ij k", k=16)
        pf2 = sb.tile([128, 512], F16)
        pf2v = pf2[:].rearrange("p (ij k) -> p ij k", k=8)
        nc.gpsimd.tensor_tensor(
            out=pf1v[:, 0:8, :], in0=ppk[:, 0:8, 0:16],
            in1=ppk[:, 0:8, 16:32], op=mybir.AluOpType.add,
        )
        nc.gpsimd.tensor_tensor(
            out=pf2v[:, 0:8, :], in0=pf1v[:, 0:8, 0:8],
            in1=pf1v[:, 0:8, 8:16], op=mybir.AluOpType.add,
        )
        nc.vector.tensor_tensor(
            out=pf1v[:, 8:64, :], in0=ppk[:, 8:64, 0:16],
            in1=ppk[:, 8:64, 16:32], op=mybir.AluOpType.add,
        )
        nc.vector.tensor_tensor(
            out=pf2v[:, 8:64, :], in0=pf1v[:, 8:64, 0:8],
            in1=pf1v[:, 8:64, 8:16], op=mybir.AluOpType.add,
        )
        gq = sb.tile([128, 64], F16)
        with nc.allow_low_precision(reason="8-term fp16 fold of unit-norm rows"):
            nc.vector.reduce_sum(out=gq[:], in_=pf2v, axis=mybir.AxisListType.X)

        # fold q via one-hot matmul: G^T [64=(i,j), 32=b]
        psum_g = ps.tile([64, B_LOC], F32)
        nc.tensor.matmul(psum_g[:], gq[:], zbq[:, 256:288])
        gT = sb.tile([64, B_LOC], F16)
        nc.vector.tensor_copy(gT[:], psum_g[:])

        # out cols 0,1 = per-chunk sum_p exp(logit) (rows 0..31); cols 2..34 =
        # G^T in f32 (host derives logit0 and the alpha*V term from G)
        out_t = sb.tile([64, 2 + B_LOC], F32)
        nc.gpsimd.memset(out_t[B_LOC : 2 * B_LOC, 0:2], 0.0)
        nc.vector.tensor_copy(out_t[:, 2 : 2 + B_LOC], psum_g[:])

        # logits: one PSUM tile per chunk so exp(chunk0) overlaps matmul(chunk1)
        psum_X0 = ps.tile([B_LOC, HALF], F32)
        psum_X1 = ps.tile([B_LOC, HALF], F32)
        nc.tensor.matmul(psum_X0[:], gT[:], callT[:, 0:HALF])
        nc.tensor.matmul(psum_X1[:], gT[:], callT[:, HALF : 2 * HALF])

        e0 = sb.tile([B_LOC, HALF], F16)
        e1 = sb.tile([B_LOC, HALF], F16)
        nc.scalar.activation(
            e0[:], psum_X0[:], mybir.ActivationFunctionType.Exp,
            accum_out=out_t[0:B_LOC, 0:1],
        )
        nc.scalar.activation(
            e1[:], psum_X1[:], mybir.ActivationFunctionType.Exp,
            accum_out=out_t[0:B_LOC, 1:2],
        )
        if chain is not None:
            nc.vector.tensor_copy(chain[:], out_t[0:1, 0:1])
        nc.sync.dma_start(out=out_d[:], in_=out_t[:])


def _build(nloop=1):
    key = ("nc", nloop)
    if key in _cache:
        return _cache[key]
    nc = bacc.Bacc(
        "TRN2",
        target_bir_lowering=False,
        debug=False,
        enable_asserts=False,
        num_devices=N_CORES,
    )
    zbq_d = nc.dram_tensor("zbq", [128, 288], F16, kind="ExternalInput").ap()
    callT_d = nc.dram_tensor("callT", [64, 2 * HALF], F16, kind="ExternalInput").ap()
    out_d = nc.dram_tensor("out_t", [64, 2 + B_LOC], F32, kind="ExternalOutput").ap()
    with tile.TileContext(nc) as tc:
        if nloop == 1:
            _kernel_body(tc, out_d, zbq_d, callT_d)
        else:
            with tc.tile_pool(name="chain", bufs=1) as cp:
                chain = cp.tile([1, 1], F32)
                for i in range(nloop):
                    _kernel_body(
                        tc, out_d, zbq_d, callT_d, chain=chain, first=(i == 0)
                    )
    nc.compile()
    _cache[key] = nc
    return nc


def _coeffs(perm_index):
    """callT [64, 1000] fp16: rows (i,j), col p = -0.5*C_p.  All entries are
    small ints or halves — exact in fp16.  Also returns -0.5*C_0 and 0.25*C1
    (f64) for the host-side logit0 / alpha*V terms."""
    perm = np.asarray(perm_index, dtype=np.int64).reshape(NUM_PERMS, T)
    E = (perm[:, :, None] == np.arange(T)[None, None, :]).astype(np.float32)
    M = E[:, 0:6] - 2.0 * E[:, 1:7] + E[:, 2:8]  # [P, 6, 8]
    C = np.einsum("pri,prj->pij", M, M)  # [P, 8, 8]
    D1 = (np.eye(T, k=1) - np.eye(T))[: T - 1]
    C1 = (D1.T @ D1).astype(np.float64)
    callT = (-0.5 * C).reshape(NUM_PERMS, 64).T.astype(np.float16)
    return np.ascontiguousarray(callT), (-0.5 * C[0]).astype(np.float64), 0.25 * C1


def _in_maps(Z, perm_index):
    callT, _, _ = _coeffs(perm_index)
    q4 = np.repeat(np.eye(B_LOC, dtype=np.float16), 4, axis=0)  # [128, 32]
    Zf = np.asarray(Z, dtype=np.float32).reshape(B, T, 4, 32)
    in_maps = []
    for c in range(N_CORES):
        zb = (
            np.ascontiguousarray(Zf[c * B_LOC : (c + 1) * B_LOC].transpose(0, 2, 1, 3))
            .reshape(128, 256)
            .astype(np.float16)
        )
        zbq = np.concatenate([zb, q4], axis=1)  # [128, 288]
        in_maps.append({"zbq": zbq, "callT": callT})
    return in_maps


def kernel(Z, perm_index, _trace=False):
    nc = _build()
    in_maps = _in_maps(Z, perm_index)
    _, c0, c1q = _coeffs(perm_index)
    res = run_bass_kernel_spmd(
        nc, in_maps, core_ids=list(range(N_CORES)), trace=_trace
    )
    total = np.float64(0.0)
    for r in res.results:
        o = np.asarray(r["out_t"], dtype=np.float64)
        s = o[0:B_LOC, 0] + o[0:B_LOC, 1]
        # G^T [64=(i,j), 32=b] in f32: host computes logit0 and alpha*V
        gT = o[:, 2 : 2 + B_LOC]
        logit0 = c0.reshape(64) @ gT
        aV = c1q.reshape(64) @ gT
        total += np.sum(np.log(s) - logit0 + aV)
    out = np.array(total / B, dtype=np.float32)
    if _trace:
        return out, res
    return out


# revision 7
# speedup vs baseline: 64734.7413x; 1.6086x over previous
"""GrwSmoothingLoss on 8 Trainium2 NeuronCores — v2.

Math: logits[b,p] = -0.5*||diff2(Z_b[perm_p])||^2 = <G_b, -0.5*C_p> where
G_b = Z_b Z_b^T (8x8 Gram) and C_p = M_p^T M_p with M_p the permuted
second-difference matrix.  alpha*V_b = <G_b, 0.25*C1>.  C_p entries are
small integers, so the coefficient matrix callT = [-0.5*C_p | 0.25*C1] is
built on host from perm_index and is EXACT in fp16.

Per core (32 batches, data-parallel over B):
  DMA in: zbq [128,288] fp16 (rows (b,q): Z[b,:,q*32:(q+1)*32] as (t,k) cols
          0..255, one-hot q-fold matrix cols 256..287) and callT [64,1002]
          fp16 (two 501-col chunks: 500 perm cols + the alpha*V col).
  Gram:   pair products on DVE (fp16 2x) + Pool slice, two binary k-folds,
          one TensorReduce -> gq [128,64]; PE folds q via the one-hot matmul
          -> G^T [64,32].
  Logits: two 501-col fp16 matmuls -> PSUM [64,501] (chunk c in partitions
          c*32..c*32+32).  One Exp over [64,500] with accum_out -> partial
          sums of exp(logit).  Host does ln / means (fp64).
"""

import numpy as np

import concourse.bacc as bacc
import concourse.bass as bass
import concourse.mybir as mybir
import concourse.tile as tile
from concourse.bass_utils import run_bass_kernel_spmd

B, T, K = 256, 8, 128
NUM_PERMS = 1000
ALPHA = 0.5
N_CORES = 8
B_LOC = B // N_CORES  # 32
HALF = NUM_PERMS // 2  # 500
CCOL = HALF + 1  # 501 cols per chunk (500 perms + aV col)
F32 = mybir.dt.float32
F16 = mybir.dt.float16

_cache = {}


def _kernel_body(tc, out_d, zbq_d, callT_d, chain=None, first=True):
    nc = tc.nc
    with (
        tc.tile_pool(name="sb", bufs=1) as sb,
        tc.tile_pool(name="ps", bufs=1, space="PSUM") as ps,
    ):
        if first:
            # dummy Exp to pull the activation table load into the DMA window
            dm0 = sb.tile([1, 1], F32)
            dm1 = sb.tile([1, 1], F32)
            nc.gpsimd.memset(dm0[:], 0.0)
            nc.scalar.activation(dm1[:], dm0[:], mybir.ActivationFunctionType.Exp)

        zbq = sb.tile([128, 288], F16)
        if chain is not None and not first:
            # serialize bench iterations: WAW on zbq forces this iter's DMA
            # to wait for the previous iter's tail
            nc.vector.tensor_copy(zbq[0:1, 0:1], chain[:])
        callT = sb.tile([64, 2 * HALF], F16)
        nc.sync.dma_start(out=zbq[:], in_=zbq_d[:])
        nc.sync.dma_start(out=callT[:], in_=callT_d[:])

        zv = zbq[:, 0:256].rearrange("p (t k) -> p t k", t=8)

        # pair products pp[(b,q),(i,j,k)] = z_i[k] * z_j[k], fp16.
        # Pool (3.7x slower/col than DVE-fp16) takes the i in {0,1} block as
        # one op plus the matching fold1 slice; DVE owns the rest + the
        # X-reduce (DVE-only hardware).
        pp = sb.tile([128, 2048], F16)
        ppv = pp[:].rearrange("p (i j k) -> p i j k", i=8, j=8)
        nc.gpsimd.tensor_tensor(
            out=ppv[:, 0:1, :, :],
            in0=zv[:, 0:1, :].unsqueeze(2).broadcast_to([128, 1, 8, 32]),
            in1=zv.unsqueeze(1).broadcast_to([128, 1, 8, 32]),
            op=mybir.AluOpType.mult,
        )
        nc.vector.tensor_tensor(
            out=ppv[:, 1:8, :, :],
            in0=zv[:, 1:8, :].unsqueeze(2).broadcast_to([128, 7, 8, 32]),
            in1=zv.unsqueeze(1).broadcast_to([128, 7, 8, 32]),
            op=mybir.AluOpType.mult,
        )

        # binary k-folds 32 -> 16 -> 8, then reduce -> gq [128, 64]
        ppk = pp[:].rearrange("p (ij k) -> p ij k", k=32)
        pf1 = sb.tile([128, 1024], F16)
        pf1v = pf1[:].rearrange("p (ij k) -> p ij k", k=16)
        pf2 = sb.tile([128, 512], F16)
        pf2v = pf2[:].rearrange("p (ij k) -> p ij k", k=8)
        nc.gpsimd.tensor_tensor(
            out=pf1v[:, 0:8, :], in0=ppk[:, 0:8, 0:16],
            in1=ppk[:, 0:8, 16:32], op=mybir.AluOpType.add,
        )
        nc.gpsimd.tensor_tensor(
            out=pf2v[:, 0:8, :], in0=pf1v[:, 0:8, 0:8],
            in1=pf1v[:, 0:8, 8:16], op=mybir.AluOpType.add,
        )
        # Pool also folds the i=7 tail (DVE's products finish before Pool
        # would otherwise idle), shrinking the DVE fold range to ij 8..56
        nc.gpsimd.tensor_tensor(
            out=pf1v[:, 56:64, :], in0=ppk[:, 56:64, 0:16],
            in1=ppk[:, 56:64, 16:32], op=mybir.AluOpType.add,
        )
        nc.gpsimd.tensor_tensor(
            out=pf2v[:, 56:64, :], in0=pf1v[:, 56:64, 0:8],
            in1=pf1v[:, 56:64, 8:16], op=mybir.AluOpType.add,
        )
        nc.vector.tensor_tensor(
            out=pf1v[:, 8:56, :], in0=ppk[:, 8:56, 0:16],
            in1=ppk[:, 8:56, 16:32], op=mybir.AluOpType.add,
        )
        nc.vector.tensor_tensor(
            out=pf2v[:, 8:56, :], in0=pf1v[:, 8:56, 0:8],
            in1=pf1v[:, 8:56, 8:16], op=mybir.AluOpType.add,
        )
        gq = sb.tile([128, 64], F16)
        with nc.allow_low_precision(reason="8-term fp16 fold of unit-norm rows"):
            nc.vector.reduce_sum(out=gq[:], in_=pf2v, axis=mybir.AxisListType.X)

        # fold q via one-hot matmul: G^T [64=(i,j), 32=b]
        psum_g = ps.tile([64, B_LOC], F32)
        nc.tensor.matmul(psum_g[:], gq[:], zbq[:, 256:288])
        gT = sb.tile([64, B_LOC], F16)
        nc.vector.tensor_copy(gT[:], psum_g[:])

        # out cols 0,1 = per-chunk sum_p exp(logit) (rows 0..31); cols 2..34 =
        # G^T in f32 (host derives logit0 and the alpha*V term from G)
        out_t = sb.tile([64, 2 + B_LOC], F32)
        nc.gpsimd.memset(out_t[B_LOC : 2 * B_LOC, 0:2], 0.0)
        nc.vector.tensor_copy(out_t[:, 2 : 2 + B_LOC], psum_g[:])

        # logits: one PSUM tile per chunk so exp(chunk0) overlaps matmul(chunk1)
        psum_X0 = ps.tile([B_LOC, HALF], F32)
        psum_X1 = ps.tile([B_LOC, HALF], F32)
        nc.tensor.matmul(psum_X0[:], gT[:], callT[:, 0:HALF])
        nc.tensor.matmul(psum_X1[:], gT[:], callT[:, HALF : 2 * HALF])

        e0 = sb.tile([B_LOC, HALF], F16)
        e1 = sb.tile([B_LOC, HALF], F16)
        nc.scalar.activation(
            e0[:], psum_X0[:], mybir.ActivationFunctionType.Exp,
            accum_out=out_t[0:B_LOC, 0:1],
        )
        nc.scalar.activation(
            e1[:], psum_X1[:], mybir.ActivationFunctionType.Exp,
            accum_out=out_t[0:B_LOC, 1:2],
        )
        if chain is not None:
            nc.vector.tensor_copy(chain[:], out_t[0:1, 0:1])
        nc.sync.dma_start(out=out_d[:], in_=out_t[:])


def _build(nloop=1):
    key = ("nc", nloop)
    if key in _cache:
        return _cache[key]
    nc = bacc.Bacc(
        "TRN2",
        target_bir_lowering=False,
        debug=False,
        enable_asserts=False,
        num_devices=N_CORES,
    )
    zbq_d = nc.dram_tensor("zbq", [128, 288], F16, kind="ExternalInput").ap()
    callT_d = nc.dram_tensor("callT", [64, 2 * HALF], F16, kind="ExternalInput").ap()
    out_d = nc.dram_tensor("out_t", [64, 2 + B_LOC], F32, kind="ExternalOutput").ap()
    with tile.TileContext(nc) as tc:
        if nloop == 1:
            _kernel_body(tc, out_d, zbq_d, callT_d)
        else:
            with tc.tile_pool(name="chain", bufs=1) as cp:
                chain = cp.tile([1, 1], F32)
                for i in range(nloop):
                    _kernel_body(
                        tc, out_d, zbq_d, callT_d, chain=chain, first=(i == 0)
                    )
    nc.compile()
    _cache[key] = nc
    return nc


def _coeffs(perm_index):
    """callT [64, 1000] fp16: rows (i,j), col p = -0.5*C_p.  All entries are
    small ints or halves — exact in fp16.  Also returns -0.5*C_0 and 0.25*C1
    (f64) for the host-side logit0 / alpha*V terms."""
    perm = np.asarray(perm_index, dtype=np.int64).reshape(NUM_PERMS, T)
    E = (perm[:, :, None] == np.arange(T)[None, None, :]).astype(np.float32)
    M = E[:, 0:6] - 2.0 * E[:, 1:7] + E[:, 2:8]  # [P, 6, 8]
    C = np.einsum("pri,prj->pij", M, M)  # [P, 8, 8]
    D1 = (np.eye(T, k=1) - np.eye(T))[: T - 1]
    C1 = (D1.T @ D1).astype(np.float64)
    callT = (-0.5 * C).reshape(NUM_PERMS, 64).T.astype(np.float16)
    return np.ascontiguousarray(callT), (-0.5 * C[0]).astype(np.float64), 0.25 * C1


def _in_maps(Z, perm_index):
    callT, _, _ = _coeffs(perm_index)
    q4 = np.repeat(np.eye(B_LOC, dtype=np.float16), 4, axis=0)  # [128, 32]
    Zf = np.asarray(Z, dtype=np.float32).reshape(B, T, 4, 32)
    in_maps = []
    for c in range(N_CORES):
        zb = (
            np.ascontiguousarray(Zf[c * B_LOC : (c + 1) * B_LOC].transpose(0, 2, 1, 3))
            .reshape(128, 256)
            .astype(np.float16)
        )
        zbq = np.concatenate([zb, q4], axis=1)  # [128, 288]
        in_maps.append({"zbq": zbq, "callT": callT})
    return in_maps


def kernel(Z, perm_index, _trace=False):
    nc = _build()
    in_maps = _in_maps(Z, perm_index)
    _, c0, c1q = _coeffs(perm_index)
    res = run_bass_kernel_spmd(
        nc, in_maps, core_ids=list(range(N_CORES)), trace=_trace
    )
    total = np.float64(0.0)
    for r in res.results:
        o = np.asarray(r["out_t"], dtype=np.float64)
        s = o[0:B_LOC, 0] + o[0:B_LOC, 1]
        # G^T [64=(i,j), 32=b] in f32: host computes logit0 and alpha*V
        gT = o[:, 2 : 2 + B_LOC]
        logit0 = c0.reshape(64) @ gT
        aV = c1q.reshape(64) @ gT
        total += np.sum(np.log(s) - logit0 + aV)
    out = np.array(total / B, dtype=np.float32)
    if _trace:
        return out, res
    return out
